# revision 2
# baseline (speedup 1.0000x reference)
"""Trainium2 Bass kernel for nn_MultiHeadAttention_48919677502060.

Head-sharded (tensor-parallel) multi-head attention with RoPE and key-padding
mask across 8 NeuronCores: each core computes 2 of the 16 heads end-to-end
(QKV projection, RoPE, masked SDPA) plus its row-shard of the output
projection; the host sums the 8 partial outputs (the "all-reduce") and adds
the bias.

Self-contained: hardcodes shapes B=2, L=2048, C=1024, H=16, D=64.
"""

import sys

sys.path.insert(0, "/opt/trn_rl_repo")

import numpy as np

B, L, C, H = 2, 2048, 1024, 16
D = C // H  # 64
THETA = 50000.0
NCORES = 8
HPC = H // NCORES  # heads per core = 2
DH = HPC * D  # channels per core = 128
BL = B * L  # 4096

_CACHE = {}


def _build_nc():
    import concourse.bacc as bacc
    import concourse.mybir as mybir
    from concourse import tile
    from concourse.masks import make_identity

    f32 = mybir.dt.float32
    bf16 = mybir.dt.bfloat16
    AF = mybir.ActivationFunctionType
    OP = mybir.AluOpType

    nc = bacc.Bacc("TRN2", target_bir_lowering=False, debug=False,
                   num_devices=NCORES)

    x_e = nc.declare_dram_parameter("x", [BL, C], f32, isOutput=False)
    wqt_e = nc.declare_dram_parameter("wqt", [C, DH], f32, isOutput=False)
    wkt_e = nc.declare_dram_parameter("wkt", [C, DH], f32, isOutput=False)
    wvt_e = nc.declare_dram_parameter("wvt", [C, DH], f32, isOutput=False)
    wot_e = nc.declare_dram_parameter("wot", [DH, C], f32, isOutput=False)
    cost_e = nc.declare_dram_parameter("cost", [128, L], f32, isOutput=False)
    sina_e = nc.declare_dram_parameter("sina", [128, L], f32, isOutput=False)
    maskc_e = nc.declare_dram_parameter("maskc", [128, B * 16], f32,
                                        isOutput=False)
    y_e = nc.declare_dram_parameter("y", [BL, C], f32, isOutput=True)

    with tile.TileContext(nc) as tc:
        with (
            tc.tile_pool(name="consts", bufs=1) as consts,
            tc.tile_pool(name="dram", bufs=2, space="DRAM") as dpool,
            tc.tile_pool(name="xrows", bufs=2) as xrp,
            tc.tile_pool(name="work", bufs=2) as wp,
            tc.tile_pool(name="et", bufs=3) as ep,
            tc.tile_pool(name="psQK", bufs=2, space="PSUM") as pQK,
            tc.tile_pool(name="psV", bufs=2, space="PSUM") as pVs,
            tc.tile_pool(name="psO", bufs=1, space="PSUM") as pO,
        ):
            # ---- constants ----
            wq_sb = consts.tile([128, 8, DH], bf16)
            wk_sb = consts.tile([128, 8, DH], bf16)
            wv_sb = consts.tile([128, 8, DH], bf16)
            wo_sb = consts.tile([128, C], bf16)
            nc.gpsimd.dma_start(out=wq_sb[:],
                                in_=wqt_e.rearrange("(t p) m -> p t m", p=128))
            nc.gpsimd.dma_start(out=wk_sb[:],
                                in_=wkt_e.rearrange("(t p) m -> p t m", p=128))
            nc.gpsimd.dma_start(out=wv_sb[:],
                                in_=wvt_e.rearrange("(t p) m -> p t m", p=128))
            nc.gpsimd.dma_start(out=wo_sb[:], in_=wot_e[:])
            cost_sb = consts.tile([128, L], f32)
            sina_sb = consts.tile([128, L], f32)
            maskc_sb = consts.tile([128, B * 16], f32)
            nc.sync.dma_start(out=cost_sb[:], in_=cost_e[:])
            nc.sync.dma_start(out=sina_sb[:], in_=sina_e[:])
            nc.sync.dma_start(out=maskc_sb[:], in_=maskc_e[:])
            ident = consts.tile([128, 128], bf16)
            make_identity(nc, ident[:])

            # ---- persistent activations ----
            xT = consts.tile([128, 8, BL], bf16)     # x^T per c-tile
            QTr = consts.tile([128, BL], bf16)       # Q^T after RoPE (2 heads)
            KTr = consts.tile([128, BL], bf16)
            Vb = consts.tile([128, B * HPC * 16, D + 1], bf16)  # [s,(V|1)]
            OTn = consts.tile([128, B, L], bf16)     # normalized O^T

            # ---- stage B: x -> bf16 -> x^T (via DRAM bounce + DMA transpose)
            for qt in range(4):
                xr = xrp.tile([128, 8, 1024], bf16, tag="xr")
                nc.gpsimd.dma_start(
                    out=xr[:],
                    in_=x_e[qt * 1024:(qt + 1) * 1024, :]
                    .rearrange("(t p) c -> p t c", p=128))
                xbf = dpool.tile([1024, C], bf16, tag="xbf")
                nc.sync.dma_start(
                    out=xbf[:].rearrange("(t p) c -> p t c", p=128), in_=xr[:])
                for cb in range(8):
                    nc.sync.dma_start(
                        out=xT[:, cb, qt * 1024:(qt + 1) * 1024],
                        in_=xbf[:, cb * 128:(cb + 1) * 128],
                        transpose=True)

            # ---- stage C: QKV projections + RoPE + V build ----
            for n in range(8):  # chunks of 512 over B*L
                b, lc = n // 4, n % 4
                ps = pQK.tile([128, 1024], f32, tag="qk")  # Q | K halves
                psv = pVs.tile([128, 512], f32, tag="v")
                for kt in range(8):
                    st, sp = (kt == 0), (kt == 7)
                    rhs = xT[:, kt, n * 512:(n + 1) * 512]
                    nc.tensor.matmul(ps[:, 0:512], wq_sb[:, kt, :], rhs,
                                     start=st, stop=sp)
                    nc.tensor.matmul(ps[:, 512:1024], wk_sb[:, kt, :], rhs,
                                     start=st, stop=sp)
                    nc.tensor.matmul(psv[:], wv_sb[:, kt, :], rhs,
                                     start=st, stop=sp)
                cc = cost_sb[:, lc * 512:(lc + 1) * 512]
                ss = sina_sb[:, lc * 512:(lc + 1) * 512]
                for which, base in (("q", 0), ("k", 512)):
                    src = ps[:, base:base + 512]
                    dst = (QTr if which == "q" else KTr)[:, n * 512:(n + 1) * 512]
                    tmp = wp.tile([128, 512], f32, tag="tmp")
                    rot = wp.tile([128, 512], f32, tag="rot")
                    nc.vector.tensor_tensor(tmp[:], src, cc, op=OP.mult)
                    for r0, s0 in ((0, 32), (32, 0), (64, 96), (96, 64)):
                        nc.vector.tensor_tensor(
                            rot[r0:r0 + 32, :],
                            ps[s0:s0 + 32, base:base + 512],
                            ss[r0:r0 + 32, :], op=OP.mult)
                    nc.vector.tensor_tensor(dst, tmp[:], rot[:], op=OP.add)
                # V: transpose psv into per-(b,h,st) [s, D|1] blocks
                vsb = wp.tile([128, 512], bf16, tag="vsb")
                nc.vector.tensor_copy(vsb[:], psv[:])
                for sb in range(4):
                    st_loc = lc * 4 + sb  # s-tile within batch b
                    mcol = maskc_sb[:, b * 16 + st_loc:b * 16 + st_loc + 1]
                    pvt = pVs.tile([128, 128], bf16, tag="v")
                    nc.tensor.transpose(pvt[:], vsb[:, sb * 128:(sb + 1) * 128],
                                        ident[:])
                    for h in range(HPC):
                        blk = (b * HPC + h) * 16 + st_loc
                        nc.vector.tensor_scalar_mul(
                            Vb[:, blk, 0:D], pvt[:, h * D:(h + 1) * D], mcol)
                        nc.vector.tensor_copy(Vb[:, blk, D:D + 1], mcol)

            # ---- stage D: attention per (b, head, l-half) ----
            for b in range(B):
                for h in range(HPC):
                    hr = slice(h * D, (h + 1) * D)
                    for lh in range(2):
                        l0 = b * L + lh * 1024
                        pso = pO.tile([128, 1024], f32, tag="o")
                        for st in range(16):
                            pss = pQK.tile([128, 1024], f32, tag="qk")
                            lhsT = KTr[hr, b * L + st * 128:b * L + (st + 1) * 128]
                            nc.tensor.matmul(pss[:, 0:512], lhsT,
                                             QTr[hr, l0:l0 + 512],
                                             start=True, stop=True)
                            nc.tensor.matmul(pss[:, 512:1024], lhsT,
                                             QTr[hr, l0 + 512:l0 + 1024],
                                             start=True, stop=True)
                            et = ep.tile([128, 1024], bf16, tag="et")
                            nc.scalar.activation(et[:], pss[:], AF.Exp,
                                                 scale=float(1.0 / np.sqrt(D)))
                            vblk = Vb[:, (b * HPC + h) * 16 + st, :]
                            st_f, sp_f = (st == 0), (st == 15)
                            nc.tensor.matmul(pso[0:D + 1, 0:512], vblk,
                                             et[:, 0:512], start=st_f, stop=sp_f)
                            nc.tensor.matmul(pso[0:D + 1, 512:1024], vblk,
                                             et[:, 512:1024], start=st_f,
                                             stop=sp_f)
                        rzt = wp.tile([1, 1024], f32, tag="rzt")
                        nc.vector.reciprocal(rzt[:], pso[D:D + 1, :])
                        rzb = wp.tile([64, 1024], f32, tag="rzb")
                        nc.gpsimd.partition_broadcast(rzb[:], rzt[:])
                        nc.vector.tensor_tensor(
                            OTn[hr, b, lh * 1024:(lh + 1) * 1024],
                            pso[0:D, :], rzb[:], op=OP.mult)

            # ---- stage E: output projection (row shard of Wo) ----
            for b in range(B):
                for m in range(16):
                    psy = pQK.tile([128, 1024], f32, tag="qk")
                    lhsT = OTn[:, b, m * 128:(m + 1) * 128]
                    nc.tensor.matmul(psy[:, 0:512], lhsT, wo_sb[:, 0:512],
                                     start=True, stop=True)
                    nc.tensor.matmul(psy[:, 512:1024], lhsT, wo_sb[:, 512:1024],
                                     start=True, stop=True)
                    ysb = wp.tile([128, 1024], f32, tag="ysb")
                    nc.scalar.activation(ysb[:, 0:512], psy[:, 0:512], AF.Copy)
                    nc.vector.tensor_copy(ysb[:, 512:1024], psy[:, 512:1024])
                    nc.sync.dma_start(
                        out=y_e[b * L + m * 128:b * L + (m + 1) * 128, :],
                        in_=ysb[:])

    nc.compile()
    return nc


def _host_constants():
    inv_freq = 1.0 / (THETA ** (np.arange(0, D, 2, dtype=np.float64) / D))
    pos = np.arange(L, dtype=np.float64)[:, None] * inv_freq[None, :]  # [L,32]
    pos = np.concatenate([pos, pos], axis=1)  # [L, 64]
    cos_t = np.cos(pos).T.astype(np.float32)  # [64, L]
    sin_t = np.sin(pos).T.astype(np.float32)
    cost = np.concatenate([cos_t, cos_t], axis=0)  # [128, L] (2 heads)
    sina_h = np.concatenate([-sin_t[0:32], sin_t[32:64]], axis=0)
    sina = np.concatenate([sina_h, sina_h], axis=0)  # [128, L]
    return np.ascontiguousarray(cost), np.ascontiguousarray(sina)


def kernel(q, mask, Wq, Wk, Wv, Wo, bo):
    q = np.asarray(q, dtype=np.float32)
    mask = np.asarray(mask)
    Wq, Wk, Wv, Wo = (np.asarray(w, dtype=np.float32) for w in (Wq, Wk, Wv, Wo))
    bo = np.asarray(bo, dtype=np.float32)

    if "nc" not in _CACHE:
        _CACHE["nc"] = _build_nc()
    nc = _CACHE["nc"]

    from concourse.bass_utils import run_bass_kernel_spmd

    cost, sina = _host_constants()
    x = np.ascontiguousarray(q.reshape(BL, C))
    maskf = mask.astype(np.float32)  # [B, L] 0/1
    maskc = np.ascontiguousarray(
        maskf.reshape(B * 16, 128).T)  # [128, B*16]

    in_maps = []
    for c in range(NCORES):
        rows = slice(c * DH, (c + 1) * DH)
        in_maps.append({
            "x": x,
            "wqt": np.ascontiguousarray(Wq[rows, :].T),
            "wkt": np.ascontiguousarray(Wk[rows, :].T),
            "wvt": np.ascontiguousarray(Wv[rows, :].T),
            "wot": np.ascontiguousarray(Wo[:, rows].T),
            "cost": cost,
            "sina": sina,
            "maskc": maskc,
        })

    res = run_bass_kernel_spmd(nc, in_maps, list(range(NCORES)))
    out = np.zeros((BL, C), dtype=np.float32)
    for c in range(NCORES):
        out += res.results[c]["y"]
    out += bo[None, :]
    return out.reshape(B, L, C)


# revision 12
# speedup vs baseline: 1.3230x; 1.3230x over previous
"""Trainium2 Bass kernel for nn_MultiHeadAttention_48919677502060.

Head-sharded (tensor-parallel) multi-head attention with RoPE and key-padding
mask across 8 NeuronCores: each core computes 2 of the 16 heads end-to-end
(QKV projection, RoPE, masked SDPA) plus its row-shard of the output
projection; the host sums the 8 partial outputs (the "all-reduce") and adds
the bias.

Self-contained: hardcodes shapes B=2, L=2048, C=1024, H=16, D=64.
"""

import sys

sys.path.insert(0, "/opt/trn_rl_repo")

import numpy as np

B, L, C, H = 2, 2048, 1024, 16
D = C // H  # 64
THETA = 50000.0
NCORES = 8
HPC = H // NCORES  # heads per core = 2
DH = HPC * D  # channels per core = 128
BL = B * L  # 4096

_CACHE = {}


def _build_nc():
    import concourse.bacc as bacc
    import concourse.mybir as mybir
    from concourse import tile
    from concourse.masks import make_identity

    f32 = mybir.dt.float32
    bf16 = mybir.dt.bfloat16
    AF = mybir.ActivationFunctionType
    OP = mybir.AluOpType

    nc = bacc.Bacc("TRN2", target_bir_lowering=False, debug=False,
                   num_devices=NCORES)

    x_e = nc.declare_dram_parameter("x", [BL, C], f32, isOutput=False)
    wqt_e = nc.declare_dram_parameter("wqt", [C, DH], f32, isOutput=False)
    wkt_e = nc.declare_dram_parameter("wkt", [C, DH], f32, isOutput=False)
    wvt_e = nc.declare_dram_parameter("wvt", [C, DH], f32, isOutput=False)
    wot_e = nc.declare_dram_parameter("wot", [DH, C], f32, isOutput=False)
    cost_e = nc.declare_dram_parameter("cost", [128, L], f32, isOutput=False)
    sina_e = nc.declare_dram_parameter("sina", [128, L], f32, isOutput=False)
    rotm_e = nc.declare_dram_parameter("rotm", [128, 128], f32, isOutput=False)
    maskc_e = nc.declare_dram_parameter("maskc", [128, B * 16], f32,
                                        isOutput=False)
    y_e = nc.declare_dram_parameter("y", [BL, C], f32, isOutput=True)

    with tile.TileContext(nc) as tc:
        with (
            tc.tile_pool(name="consts", bufs=1) as consts,
            tc.tile_pool(name="dram", bufs=2, space="DRAM") as dpool,
            tc.tile_pool(name="work", bufs=2) as wp,
            tc.tile_pool(name="et", bufs=3) as ep,
            tc.tile_pool(name="psQK", bufs=2, space="PSUM") as pQK,
            tc.tile_pool(name="psV", bufs=2, space="PSUM") as pVs,
            tc.tile_pool(name="psO", bufs=1, space="PSUM") as pO,
        ):
            # ---- constants ----
            wq_sb = consts.tile([128, 8, DH], bf16)
            wk_sb = consts.tile([128, 8, DH], bf16)
            wv_sb = consts.tile([128, 8, DH], bf16)
            wo_sb = consts.tile([128, C], bf16)
            nc.gpsimd.dma_start(out=wq_sb[:],
                                in_=wqt_e.rearrange("(t p) m -> p t m", p=128))
            nc.gpsimd.dma_start(out=wk_sb[:],
                                in_=wkt_e.rearrange("(t p) m -> p t m", p=128))
            nc.gpsimd.dma_start(out=wv_sb[:],
                                in_=wvt_e.rearrange("(t p) m -> p t m", p=128))
            nc.gpsimd.dma_start(out=wo_sb[:], in_=wot_e[:])
            cost_sb = consts.tile([128, L], f32)
            sina_sb = consts.tile([128, L], f32)
            maskc_sb = consts.tile([128, B * 16], f32)
            nc.scalar.dma_start(out=cost_sb[:], in_=cost_e[:])
            nc.scalar.dma_start(out=sina_sb[:], in_=sina_e[:])
            nc.scalar.dma_start(out=maskc_sb[:], in_=maskc_e[:])
            ident = consts.tile([128, 128], bf16)
            make_identity(nc, ident[:])
            rotm_sb = consts.tile([128, 128], bf16)
            nc.gpsimd.dma_start(out=rotm_sb[:], in_=rotm_e[:])

            # ---- persistent activations ----
            xT = consts.tile([128, 8, BL], bf16)     # x^T per c-tile
            QTr = consts.tile([128, BL], bf16)       # Q^T after RoPE (2 heads)
            KTr = consts.tile([128, BL], bf16)
            Vb = consts.tile([128, B * HPC * 16, D + 1], bf16)  # [s,(V|1)]
            OTn = consts.tile([128, B, L], bf16)     # normalized O^T

            # ---- stage B: x -> bf16 (DRAM->DRAM cast) -> x^T (DMA transpose)
            for qt in range(4):
                xbf = dpool.tile([1024, C], bf16, tag="xbf")
                nc.gpsimd.dma_start(
                    out=xbf[:], in_=x_e[qt * 1024:(qt + 1) * 1024, :])
                for cb in range(8):
                    nc.sync.dma_start(
                        out=xT[:, cb, qt * 1024:(qt + 1) * 1024],
                        in_=xbf[:, cb * 128:(cb + 1) * 128],
                        transpose=True)

            # ---- stage C: QKV projections + RoPE + V build ----
            for n in range(8):  # chunks of 512 over B*L
                b, lc = n // 4, n % 4
                ps = pQK.tile([128, 1024], f32, tag="qk")  # Q | K halves
                psv = pVs.tile([128, 512], f32, tag="v")
                for kt in range(8):
                    st, sp = (kt == 0), (kt == 7)
                    rhs = xT[:, kt, n * 512:(n + 1) * 512]
                    nc.tensor.matmul(ps[:, 0:512], wq_sb[:, kt, :], rhs,
                                     start=st, stop=sp)
                    nc.tensor.matmul(ps[:, 512:1024], wk_sb[:, kt, :], rhs,
                                     start=st, stop=sp)
                    nc.tensor.matmul(psv[:], wv_sb[:, kt, :], rhs,
                                     start=st, stop=sp)
                cc = cost_sb[:, lc * 512:(lc + 1) * 512]
                ss = sina_sb[:, lc * 512:(lc + 1) * 512]
                # drain QK psum through ACT (fast psum port, frees the bank)
                qkb = wp.tile([128, 1024], bf16, tag="qkb")
                nc.scalar.activation(qkb[:], ps[:], AF.Copy)
                # rotate_half via PE: rot = R @ qkb (signs folded into R)
                psr = pQK.tile([128, 1024], f32, tag="qk")
                nc.tensor.matmul(psr[:, 0:512], rotm_sb[:], qkb[:, 0:512],
                                 start=True, stop=True)
                nc.tensor.matmul(psr[:, 512:1024], rotm_sb[:], qkb[:, 512:1024],
                                 start=True, stop=True)
                for which, base in (("q", 0), ("k", 512)):
                    dst = (QTr if which == "q" else KTr)[:, n * 512:(n + 1) * 512]
                    tmp = wp.tile([128, 512], f32, tag="tmp")
                    nc.vector.tensor_tensor(tmp[:], qkb[:, base:base + 512], cc,
                                            op=OP.mult)
                    rot = wp.tile([128, 512], f32, tag="rot")
                    nc.vector.tensor_tensor(rot[:], psr[:, base:base + 512], ss,
                                            op=OP.mult)
                    nc.vector.tensor_tensor(dst, tmp[:], rot[:], op=OP.add)
                # V: transpose psv into per-(b,h,st) [s, D|1] blocks
                vsb = wp.tile([128, 512], bf16, tag="vsb")
                nc.scalar.activation(vsb[:], psv[:], AF.Copy)
                for sb in range(4):
                    st_loc = lc * 4 + sb  # s-tile within batch b
                    mcol = maskc_sb[:, b * 16 + st_loc:b * 16 + st_loc + 1]
                    pvt = pVs.tile([128, 128], bf16, tag="v")
                    nc.tensor.transpose(pvt[:], vsb[:, sb * 128:(sb + 1) * 128],
                                        ident[:])
                    vts = wp.tile([128, 128], bf16, tag="vts")
                    nc.scalar.activation(vts[:], pvt[:], AF.Copy)
                    for h in range(HPC):
                        blk = (b * HPC + h) * 16 + st_loc
                        nc.vector.tensor_scalar_mul(
                            Vb[:, blk, 0:D], vts[:, h * D:(h + 1) * D], mcol)
                        nc.vector.tensor_copy(Vb[:, blk, D:D + 1], mcol)

            # ---- stage D: attention per (b, head, l-half) ----
            for b in range(B):
                for h in range(HPC):
                    hr = slice(h * D, (h + 1) * D)
                    for lh in range(2):
                        l0 = b * L + lh * 1024
                        pso = pO.tile([128, 1024], f32, tag="o")
                        for st in range(16):
                            pss = pQK.tile([128, 1024], f32, tag="qk")
                            lhsT = KTr[hr, b * L + st * 128:b * L + (st + 1) * 128]
                            nc.tensor.matmul(pss[:, 0:512], lhsT,
                                             QTr[hr, l0:l0 + 512],
                                             start=True, stop=True)
                            nc.tensor.matmul(pss[:, 512:1024], lhsT,
                                             QTr[hr, l0 + 512:l0 + 1024],
                                             start=True, stop=True)
                            et = ep.tile([128, 1024], bf16, tag="et")
                            nc.scalar.activation(et[:], pss[:], AF.Exp,
                                                 scale=float(1.0 / np.sqrt(D)))
                            vblk = Vb[:, (b * HPC + h) * 16 + st, :]
                            st_f, sp_f = (st == 0), (st == 15)
                            nc.tensor.matmul(pso[0:D + 1, 0:512], vblk,
                                             et[:, 0:512], start=st_f, stop=sp_f)
                            nc.tensor.matmul(pso[0:D + 1, 512:1024], vblk,
                                             et[:, 512:1024], start=st_f,
                                             stop=sp_f)
                        # drain O psum via ACT (frees the bank fast), then
                        # normalize out of SBUF on DVE
                        osb = wp.tile([D + 1, 1024], f32, tag="osb")
                        nc.scalar.activation(osb[:], pso[0:D + 1, :], AF.Copy)
                        rzt = wp.tile([1, 1024], f32, tag="rzt")
                        nc.vector.reciprocal(rzt[:], osb[D:D + 1, :])
                        rzb = wp.tile([64, 1024], f32, tag="rzb")
                        nc.gpsimd.partition_broadcast(rzb[:], rzt[:])
                        nc.vector.tensor_tensor(
                            OTn[hr, b, lh * 1024:(lh + 1) * 1024],
                            osb[0:D, :], rzb[:], op=OP.mult)

            # ---- stage E: output projection (row shard of Wo) ----
            for b in range(B):
                for m in range(16):
                    psy = pQK.tile([128, 1024], f32, tag="qk")
                    lhsT = OTn[:, b, m * 128:(m + 1) * 128]
                    nc.tensor.matmul(psy[:, 0:512], lhsT, wo_sb[:, 0:512],
                                     start=True, stop=True)
                    nc.tensor.matmul(psy[:, 512:1024], lhsT, wo_sb[:, 512:1024],
                                     start=True, stop=True)
                    ysb = wp.tile([128, 1024], f32, tag="ysb")
                    nc.scalar.activation(ysb[:, 0:512], psy[:, 0:512], AF.Copy)
                    nc.vector.tensor_copy(ysb[:, 512:1024], psy[:, 512:1024])
                    nc.scalar.dma_start(
                        out=y_e[b * L + m * 128:b * L + (m + 1) * 128, :],
                        in_=ysb[:])

    nc.compile()
    return nc


def _host_constants():
    inv_freq = 1.0 / (THETA ** (np.arange(0, D, 2, dtype=np.float64) / D))
    pos = np.arange(L, dtype=np.float64)[:, None] * inv_freq[None, :]  # [L,32]
    pos = np.concatenate([pos, pos], axis=1)  # [L, 64]
    cos_t = np.cos(pos).T.astype(np.float32)  # [64, L]
    sin_t = np.sin(pos).T.astype(np.float32)
    cost = np.concatenate([cos_t, cos_t], axis=0)  # [128, L] (2 heads)
    sina = np.concatenate([sin_t, sin_t], axis=0)  # [128, L] (2 heads)
    # rotate_half as a matmul: psr = rotm.T @ q gives rot(q) rows
    rotm = np.zeros((128, 128), dtype=np.float32)
    for blk in (0, 64):
        for m in range(32):
            rotm[blk + m + 32, blk + m] = -1.0
            rotm[blk + m, blk + m + 32] = 1.0
    return (np.ascontiguousarray(cost), np.ascontiguousarray(sina),
            np.ascontiguousarray(rotm))


def kernel(q, mask, Wq, Wk, Wv, Wo, bo):
    q = np.asarray(q, dtype=np.float32)
    mask = np.asarray(mask)
    Wq, Wk, Wv, Wo = (np.asarray(w, dtype=np.float32) for w in (Wq, Wk, Wv, Wo))
    bo = np.asarray(bo, dtype=np.float32)

    if "nc" not in _CACHE:
        _CACHE["nc"] = _build_nc()
    nc = _CACHE["nc"]

    from concourse.bass_utils import run_bass_kernel_spmd

    cost, sina, rotm = _host_constants()
    x = np.ascontiguousarray(q.reshape(BL, C))
    maskf = mask.astype(np.float32)  # [B, L] 0/1
    maskc = np.ascontiguousarray(
        maskf.reshape(B * 16, 128).T)  # [128, B*16]

    in_maps = []
    for c in range(NCORES):
        rows = slice(c * DH, (c + 1) * DH)
        in_maps.append({
            "x": x,
            "wqt": np.ascontiguousarray(Wq[rows, :].T),
            "wkt": np.ascontiguousarray(Wk[rows, :].T),
            "wvt": np.ascontiguousarray(Wv[rows, :].T),
            "wot": np.ascontiguousarray(Wo[:, rows].T),
            "cost": cost,
            "sina": sina,
            "rotm": rotm,
            "maskc": maskc,
        })

    res = run_bass_kernel_spmd(nc, in_maps, list(range(NCORES)))
    out = np.zeros((BL, C), dtype=np.float32)
    for c in range(NCORES):
        out += res.results[c]["y"]
    out += bo[None, :]
    return out.reshape(B, L, C)


# revision 18
# speedup vs baseline: 1.3646x; 1.0315x over previous
"""Trainium2 Bass kernel for nn_MultiHeadAttention_48919677502060.

Head-sharded (tensor-parallel) multi-head attention with RoPE and key-padding
mask across 8 NeuronCores: each core computes 2 of the 16 heads end-to-end
(QKV projection, RoPE, masked SDPA) plus its row-shard of the output
projection; the host sums the 8 partial outputs (the "all-reduce") and adds
the bias.

Self-contained: hardcodes shapes B=2, L=2048, C=1024, H=16, D=64.
"""

import sys

sys.path.insert(0, "/opt/trn_rl_repo")

import numpy as np

B, L, C, H = 2, 2048, 1024, 16
D = C // H  # 64
THETA = 50000.0
NCORES = 8
HPC = H // NCORES  # heads per core = 2
DH = HPC * D  # channels per core = 128
BL = B * L  # 4096

_CACHE = {}


def _build_nc():
    import concourse.bacc as bacc
    import concourse.mybir as mybir
    from concourse import tile
    from concourse.masks import make_identity

    f32 = mybir.dt.float32
    bf16 = mybir.dt.bfloat16
    AF = mybir.ActivationFunctionType
    OP = mybir.AluOpType

    nc = bacc.Bacc("TRN2", target_bir_lowering=False, debug=False,
                   num_devices=NCORES)

    x_e = nc.declare_dram_parameter("x", [BL, C], f32, isOutput=False)
    wqt_e = nc.declare_dram_parameter("wqt", [C, DH], f32, isOutput=False)
    wkt_e = nc.declare_dram_parameter("wkt", [C, DH], f32, isOutput=False)
    wvt_e = nc.declare_dram_parameter("wvt", [C, DH], f32, isOutput=False)
    wot_e = nc.declare_dram_parameter("wot", [DH, C], f32, isOutput=False)
    cost_e = nc.declare_dram_parameter("cost", [128, L], f32, isOutput=False)
    sina_e = nc.declare_dram_parameter("sina", [128, L], f32, isOutput=False)
    rotm_e = nc.declare_dram_parameter("rotm", [128, 128], f32, isOutput=False)
    maskc_e = nc.declare_dram_parameter("maskc", [128, B * 16], f32,
                                        isOutput=False)
    y_e = nc.declare_dram_parameter("y", [BL, C], f32, isOutput=True)

    with tile.TileContext(nc) as tc:
        with (
            tc.tile_pool(name="consts", bufs=1) as consts,
            tc.tile_pool(name="dram", bufs=2, space="DRAM") as dpool,
            tc.tile_pool(name="work", bufs=2) as wp,
            tc.tile_pool(name="et", bufs=3) as ep,
            tc.tile_pool(name="psQK", bufs=2, space="PSUM") as pQK,
            tc.tile_pool(name="psV", bufs=2, space="PSUM") as pVs,
            tc.tile_pool(name="psO", bufs=1, space="PSUM") as pO,
        ):
            # ---- constants ----
            wq_sb = consts.tile([128, 8, DH], bf16)
            wk_sb = consts.tile([128, 8, DH], bf16)
            wv_sb = consts.tile([128, 8, DH], bf16)
            wo_sb = consts.tile([128, C], bf16)
            nc.gpsimd.dma_start(out=wq_sb[:],
                                in_=wqt_e.rearrange("(t p) m -> p t m", p=128))
            nc.gpsimd.dma_start(out=wk_sb[:],
                                in_=wkt_e.rearrange("(t p) m -> p t m", p=128))
            nc.gpsimd.dma_start(out=wv_sb[:],
                                in_=wvt_e.rearrange("(t p) m -> p t m", p=128))
            nc.gpsimd.dma_start(out=wo_sb[:], in_=wot_e[:])
            cost_sb = consts.tile([128, L], f32)
            sina_sb = consts.tile([128, L], f32)
            maskc_sb = consts.tile([128, B * 16], f32)
            nc.scalar.dma_start(out=cost_sb[:], in_=cost_e[:])
            nc.scalar.dma_start(out=sina_sb[:], in_=sina_e[:])
            nc.scalar.dma_start(out=maskc_sb[:], in_=maskc_e[:])
            ident = consts.tile([128, 128], bf16)
            make_identity(nc, ident[:])
            rotm_sb = consts.tile([128, 128], bf16)
            nc.gpsimd.dma_start(out=rotm_sb[:], in_=rotm_e[:])

            # ---- persistent activations ----
            # split along producer/consumer boundaries so Tile's dependency
            # tracking lets stages overlap (deps are per-tile)
            xTq = [consts.tile([128, 8, 1024], bf16, name=f"xT{qt}",
                               tag=f"xT{qt}") for qt in range(4)]
            QTb = [consts.tile([128, L], bf16, name=f"QT{b}", tag=f"QT{b}")
                   for b in range(B)]
            KTb = [consts.tile([128, L], bf16, name=f"KT{b}", tag=f"KT{b}")
                   for b in range(B)]
            Vbb = [consts.tile([128, HPC * 16, D + 1], bf16, name=f"Vb{b}",
                               tag=f"Vb{b}") for b in range(B)]
            OTb = [consts.tile([128, L], bf16, name=f"OT{b}", tag=f"OT{b}")
                   for b in range(B)]

            # ---- stage B: x -> bf16 (DRAM->DRAM cast) -> x^T (DMA transpose)
            for qt in range(4):
                xbf = dpool.tile([1024, C], bf16, tag="xbf")
                nc.gpsimd.dma_start(
                    out=xbf[:], in_=x_e[qt * 1024:(qt + 1) * 1024, :])
                for cb in range(8):
                    nc.sync.dma_start(
                        out=xTq[qt][:, cb, :],
                        in_=xbf[:, cb * 128:(cb + 1) * 128],
                        transpose=True)

            # ---- stage C: QKV projections + RoPE + V build ----
            for n in range(8):  # chunks of 512 over B*L
                b, lc = n // 4, n % 4
                ps = pQK.tile([128, 1024], f32, tag="qk")  # Q | K halves
                psv = pVs.tile([128, 512], f32, tag="v")
                qt, qh = n // 2, n % 2
                for kt in range(8):
                    st, sp = (kt == 0), (kt == 7)
                    rhs = xTq[qt][:, kt, qh * 512:(qh + 1) * 512]
                    nc.tensor.matmul(ps[:, 0:512], wq_sb[:, kt, :], rhs,
                                     start=st, stop=sp)
                    nc.tensor.matmul(ps[:, 512:1024], wk_sb[:, kt, :], rhs,
                                     start=st, stop=sp)
                    nc.tensor.matmul(psv[:], wv_sb[:, kt, :], rhs,
                                     start=st, stop=sp)
                cc = cost_sb[:, lc * 512:(lc + 1) * 512]
                ss = sina_sb[:, lc * 512:(lc + 1) * 512]
                # drain QK psum through ACT (fast psum port, frees the bank)
                qkb = wp.tile([128, 1024], bf16, tag="qkb")
                nc.scalar.activation(qkb[:], ps[:], AF.Copy)
                # rotate_half via PE: rot = R @ qkb (signs folded into R)
                psr = pQK.tile([128, 1024], f32, tag="qk")
                nc.tensor.matmul(psr[:, 0:512], rotm_sb[:], qkb[:, 0:512],
                                 start=True, stop=True)
                nc.tensor.matmul(psr[:, 512:1024], rotm_sb[:], qkb[:, 512:1024],
                                 start=True, stop=True)
                for which, base in (("q", 0), ("k", 512)):
                    dst = (QTb if which == "q" else KTb)[b][:, lc * 512:(lc + 1) * 512]
                    tmp = wp.tile([128, 512], f32, tag="tmp")
                    nc.vector.tensor_tensor(tmp[:], qkb[:, base:base + 512], cc,
                                            op=OP.mult)
                    rot = wp.tile([128, 512], f32, tag="rot")
                    nc.vector.tensor_tensor(rot[:], psr[:, base:base + 512], ss,
                                            op=OP.mult)
                    nc.vector.tensor_tensor(dst, tmp[:], rot[:], op=OP.add)
                # V: transpose psv into per-(b,h,st) [s, D|1] blocks
                vsb = wp.tile([128, 512], bf16, tag="vsb")
                nc.scalar.activation(vsb[:], psv[:], AF.Copy)
                for sb in range(4):
                    st_loc = lc * 4 + sb  # s-tile within batch b
                    mcol = maskc_sb[:, b * 16 + st_loc:b * 16 + st_loc + 1]
                    pvt = pVs.tile([128, 128], bf16, tag="v")
                    nc.tensor.transpose(pvt[:], vsb[:, sb * 128:(sb + 1) * 128],
                                        ident[:])
                    vts = wp.tile([128, 128], bf16, tag="vts")
                    nc.scalar.activation(vts[:], pvt[:], AF.Copy)
                    for h in range(HPC):
                        blk = h * 16 + st_loc
                        nc.vector.tensor_scalar_mul(
                            Vbb[b][:, blk, 0:D],
                            vts[:, h * D:(h + 1) * D], mcol)
                        nc.vector.tensor_copy(Vbb[b][:, blk, D:D + 1], mcol)

            # ---- stage D: attention per (b, head, l-half) ----
            for b in range(B):
                for h in range(HPC):
                    hr = slice(h * D, (h + 1) * D)
                    for lh in range(2):
                        l0 = lh * 1024
                        pso = pO.tile([128, 1024], f32, tag="o")
                        for st in range(16):
                            pss = pQK.tile([128, 1024], f32, tag="qk")
                            lhsT = KTb[b][hr, st * 128:(st + 1) * 128]
                            nc.tensor.matmul(pss[:, 0:512], lhsT,
                                             QTb[b][hr, l0:l0 + 512],
                                             start=True, stop=True)
                            nc.tensor.matmul(pss[:, 512:1024], lhsT,
                                             QTb[b][hr, l0 + 512:l0 + 1024],
                                             start=True, stop=True)
                            et = ep.tile([128, 1024], bf16, tag="et")
                            nc.scalar.activation(et[:], pss[:], AF.Exp,
                                                 scale=float(1.0 / np.sqrt(D)))
                            vblk = Vbb[b][:, h * 16 + st, :]
                            st_f, sp_f = (st == 0), (st == 15)
                            nc.tensor.matmul(pso[0:D + 1, 0:512], vblk,
                                             et[:, 0:512], start=st_f, stop=sp_f)
                            nc.tensor.matmul(pso[0:D + 1, 512:1024], vblk,
                                             et[:, 512:1024], start=st_f,
                                             stop=sp_f)
                        # drain O psum via ACT (frees the bank fast).
                        # Vb col 0 is the ones/mask column, so psum row 0
                        # is Z: broadcast it wide from partition 0, wide
                        # reciprocal (a [1,N] reciprocal uses 1 of 128 DVE
                        # lanes - 6.5us), then multiply.
                        osb = wp.tile([D + 1, 1024], f32, tag="osb")
                        nc.scalar.activation(osb[:], pso[0:D + 1, :], AF.Copy)
                        # move the Z row to partition 0 (tiny SBUF->SBUF DMA;
                        # partition_broadcast reads partition 0 on HW)
                        zt = wp.tile([1, 1024], f32, tag="zt")
                        nc.sync.dma_start(out=zt[:], in_=osb[D:D + 1, :])
                        rzb = wp.tile([64, 1024], f32, tag="rzb")
                        nc.gpsimd.partition_broadcast(rzb[:], zt[:])
                        nc.vector.reciprocal(rzb[:], rzb[:])
                        nc.vector.tensor_tensor(
                            OTb[b][hr, lh * 1024:(lh + 1) * 1024],
                            osb[0:D, :], rzb[:], op=OP.mult)

            # ---- stage E: output projection (row shard of Wo) ----
            for b in range(B):
                for m in range(16):
                    psy = pQK.tile([128, 1024], f32, tag="qk")
                    lhsT = OTb[b][:, m * 128:(m + 1) * 128]
                    nc.tensor.matmul(psy[:, 0:512], lhsT, wo_sb[:, 0:512],
                                     start=True, stop=True)
                    nc.tensor.matmul(psy[:, 512:1024], lhsT, wo_sb[:, 512:1024],
                                     start=True, stop=True)
                    ysb = wp.tile([128, 1024], f32, tag="ysb")
                    nc.scalar.activation(ysb[:, 0:512], psy[:, 0:512], AF.Copy)
                    nc.vector.tensor_copy(ysb[:, 512:1024], psy[:, 512:1024])
                    nc.scalar.dma_start(
                        out=y_e[b * L + m * 128:b * L + (m + 1) * 128, :],
                        in_=ysb[:])

    nc.compile()
    return nc


def _host_constants():
    inv_freq = 1.0 / (THETA ** (np.arange(0, D, 2, dtype=np.float64) / D))
    pos = np.arange(L, dtype=np.float64)[:, None] * inv_freq[None, :]  # [L,32]
    pos = np.concatenate([pos, pos], axis=1)  # [L, 64]
    cos_t = np.cos(pos).T.astype(np.float32)  # [64, L]
    sin_t = np.sin(pos).T.astype(np.float32)
    cost = np.concatenate([cos_t, cos_t], axis=0)  # [128, L] (2 heads)
    sina = np.concatenate([sin_t, sin_t], axis=0)  # [128, L] (2 heads)
    # rotate_half as a matmul: psr = rotm.T @ q gives rot(q) rows
    rotm = np.zeros((128, 128), dtype=np.float32)
    for blk in (0, 64):
        for m in range(32):
            rotm[blk + m + 32, blk + m] = -1.0
            rotm[blk + m, blk + m + 32] = 1.0
    return (np.ascontiguousarray(cost), np.ascontiguousarray(sina),
            np.ascontiguousarray(rotm))


def kernel(q, mask, Wq, Wk, Wv, Wo, bo):
    q = np.asarray(q, dtype=np.float32)
    mask = np.asarray(mask)
    Wq, Wk, Wv, Wo = (np.asarray(w, dtype=np.float32) for w in (Wq, Wk, Wv, Wo))
    bo = np.asarray(bo, dtype=np.float32)

    if "nc" not in _CACHE:
        _CACHE["nc"] = _build_nc()
    nc = _CACHE["nc"]

    from concourse.bass_utils import run_bass_kernel_spmd

    cost, sina, rotm = _host_constants()
    x = np.ascontiguousarray(q.reshape(BL, C))
    maskf = mask.astype(np.float32)  # [B, L] 0/1
    maskc = np.ascontiguousarray(
        maskf.reshape(B * 16, 128).T)  # [128, B*16]

    in_maps = []
    for c in range(NCORES):
        rows = slice(c * DH, (c + 1) * DH)
        in_maps.append({
            "x": x,
            "wqt": np.ascontiguousarray(Wq[rows, :].T),
            "wkt": np.ascontiguousarray(Wk[rows, :].T),
            "wvt": np.ascontiguousarray(Wv[rows, :].T),
            "wot": np.ascontiguousarray(Wo[:, rows].T),
            "cost": cost,
            "sina": sina,
            "rotm": rotm,
            "maskc": maskc,
        })

    res = run_bass_kernel_spmd(nc, in_maps, list(range(NCORES)))
    out = np.zeros((BL, C), dtype=np.float32)
    for c in range(NCORES):
        out += res.results[c]["y"]
    out += bo[None, :]
    return out.reshape(B, L, C)
